# revision 1
# baseline (speedup 1.0000x reference)
"""DecoderRNN (GRU + embedding + vocab projection) Bass kernel for 8 trn2 cores.

Model (per reference):
  toks = [2, x[0..S-2]]                          (teacher forcing, S=64, B=64)
  e_s  = relu(emb[toks_s])                       (E=512, padding row 0 = 0)
  GRU: r = sig(e@Wir^T + b_ir + h@Whr^T + b_hr)
       z = sig(e@Wiz^T + b_iz + h@Whz^T + b_hz)
       n = tanh(e@Win^T + b_in + r*(h@Whn^T + b_hn))
       h' = (1-z)*n + z*h                        (H=1024)
  logits_s = h_s @ Wout^T + b_out                (V=32000)
  out = logits.transpose(1,0,2)[None]            -> (1, B, S, V) f32

Distribution: the GRU recurrence is inherently serial (per-step cross-core
sync costs more than it saves at B=64/H=1024), so every core runs the full
recurrence redundantly; the output projection is vocab-split 8 ways
(4000 columns per core) and its matmuls are statically interleaved into the
recurrence to fill the PE while the per-step gate chain (DVE/ACT) runs.

Layouts ("stacked" = batch folded into 128 partitions as two 512-wide
halves; partition p <-> (half=p//64, b=p%64)):
  psum_rz [128,1024]: cols j<512 -> gate r col 512*half+j, j>=512 -> z
  psum_hn/gin [128,512]: col j -> gate n col 512*half+j
  hT [128, 8*64] fp16: col 64k+b, partition p -> h[b, 128k+p]  (matmul lhsT)
Matmuls run fp16 (exact products, fp32 PSUM accumulate); gate arithmetic
fp32; state h kept fp32 (stacked) + fp16 (transposed, for lhsT).
"""

import sys

sys.path.insert(0, "/opt/trn_rl_repo")

import numpy as np

import concourse.bass as bass
import concourse.bacc as bacc
import concourse.mybir as mybir
import concourse.tile as tile
from concourse.bass_utils import run_bass_kernel_spmd
from concourse.masks import make_identity

FP16 = mybir.dt.float16
F32 = mybir.dt.float32
I32 = mybir.dt.int32

S, B, H, E, V = 64, 64, 1024, 512, 32000
NCORES = 8
VC = V // NCORES          # 4000 vocab cols per core
SB = S * B                # 4096
KH = H // 128             # 8 h k-chunks
KE = E // 128             # 4 e k-chunks
NN = 8                    # output n-chunks per core
NW = VC // NN             # 500 cols per n-chunk
NT = SB // 128            # 32 output row tiles

_CACHE = {}


def _build(n_steps=S, with_jobs=True):
    key = ("nc", n_steps, with_jobs)
    if key in _CACHE:
        return _CACHE[key]

    nc = bacc.Bacc("TRN2", target_bir_lowering=False, debug=False)

    def din(name, shape, dt):
        return nc.dram_tensor(name, shape, dt, kind="ExternalInput").ap()

    emb_d = din("emb_t", [V, E], FP16)
    idx_d = din("idx", [128, NT], I32)
    w_rzA_d = din("w_rzA", [128, KH, 1024], FP16)
    w_rzB_d = din("w_rzB", [128, KH, 1024], FP16)
    w_hnA_d = din("w_hnA", [128, KH, 512], FP16)
    w_hnB_d = din("w_hnB", [128, KH, 512], FP16)
    w_irzA_d = din("w_irzA", [128, KE, 1024], FP16)
    w_irzB_d = din("w_irzB", [128, KE, 1024], FP16)
    w_inA_d = din("w_inA", [128, KE, 512], FP16)
    w_inB_d = din("w_inB", [128, KE, 512], FP16)
    bias_rz_d = din("bias_rz", [128, 1024], F32)
    bias_nhh_d = din("bias_nhh", [128, 512], F32)
    bias_nih_d = din("bias_nih", [128, 512], F32)
    hT0_d = din("hT0", [128, 4, 128], FP16)
    h0st_d = din("h0st", [128, 512], F32)
    w_outT_d = din("w_outT", [128, KH, VC], FP16)
    b_out_d = din("b_out_bc", [128, VC], FP16)
    out_d = nc.dram_tensor("out", [SB, VC], F32, kind="ExternalOutput").ap()

    with tile.TileContext(nc) as tc:
        with tc.tile_pool(name="const", bufs=1) as pc, \
             tc.tile_pool(name="roll", bufs=1) as pr, \
             tc.tile_pool(name="psum", bufs=1, space="PSUM") as pp:

            # ---- constants in SBUF
            w_rzA = pc.tile([128, KH, 1024], FP16, name="w_rzA")
            w_rzB = pc.tile([128, KH, 1024], FP16, name="w_rzB")
            w_hnA = pc.tile([128, KH, 512], FP16, name="w_hnA")
            w_hnB = pc.tile([128, KH, 512], FP16, name="w_hnB")
            w_irzA = pc.tile([128, KE, 1024], FP16, name="w_irzA")
            w_irzB = pc.tile([128, KE, 1024], FP16, name="w_irzB")
            w_inA = pc.tile([128, KE, 512], FP16, name="w_inA")
            w_inB = pc.tile([128, KE, 512], FP16, name="w_inB")
            bias_rz = pc.tile([128, 1024], F32, name="bias_rz")
            bias_nhh = pc.tile([128, 512], F32, name="bias_nhh")
            bias_nih = pc.tile([128, 512], F32, name="bias_nih")
            w_outT = pc.tile([128, KH, VC], FP16, name="w_outT")
            b_out = pc.tile([128, VC], FP16, name="b_out")
            idx = pc.tile([128, NT], I32, name="idx")
            ident = pc.tile([128, 128], FP16, name="ident")

            for t, d in [(w_rzA, w_rzA_d), (w_rzB, w_rzB_d), (w_hnA, w_hnA_d),
                         (w_hnB, w_hnB_d), (w_irzA, w_irzA_d), (w_irzB, w_irzB_d),
                         (w_inA, w_inA_d), (w_inB, w_inB_d), (bias_rz, bias_rz_d),
                         (bias_nhh, bias_nhh_d), (bias_nih, bias_nih_d),
                         (w_outT, w_outT_d), (b_out, b_out_d), (idx, idx_d)]:
                nc.sync.dma_start(out=t[:], in_=d[:])
            make_identity(nc, ident[:])

            # ---- rolling state/window tiles (allocated per use, slot-rotated)
            def gather_tile(g):
                """Gather 128 token rows for tile g, then transpose into the
                eT window: eT_g[p, c, j] = e[j, 128c+p] (j in 0..127 tokens)."""
                er = pr.tile([128, E], FP16, name=f"er{g}", tag="er", bufs=3)
                nc.gpsimd.indirect_dma_start(
                    out=er[:], out_offset=None,
                    in_=emb_d[:],
                    in_offset=bass.IndirectOffsetOnAxis(ap=idx[:, g:g + 1], axis=0),
                )
                eT = pr.tile([128, KE, 128], FP16, name=f"eT{g}", tag="eT", bufs=8)
                nc.sync.dma_start_transpose(out=eT[:], in_=er[:])
                return eT

            # prime the embedding pipeline (8 tiles = 16 steps of lead)
            eT_w = {g: gather_tile(g) for g in range(min(8, (n_steps + 1) // 2))}

            hT = pr.tile([128, 4, 128], FP16, name="hT_init", tag="hT", bufs=2)
            h_st = pr.tile([128, 512], F32, name="hst_init", tag="hst", bufs=2)
            nc.sync.dma_start(out=hT[:], in_=hT0_d[:])
            nc.sync.dma_start(out=h_st[:], in_=h0st_d[:])

            hs_w = {}     # output-ready hidden tiles: t -> [128, KH, 128] fp16

            # output job list: (t, nn), 8 MMs each; emitted 4/step from s=3
            jobs = [(t, nn) for t in range(n_steps // 2) for nn in range(NN)]
            if not with_jobs:
                jobs = []
            jp = 0  # job pointer

            def emit_job(t, nn):
                ps_o = pp.tile([128, NW], F32, name=f"pso{t}_{nn}", tag="pso", bufs=3)
                hst_t = hs_w[t]
                for k in range(KH):
                    nc.tensor.matmul(
                        out=ps_o[:], lhsT=hst_t[:, k, :],
                        rhs=w_outT[:, k, nn * NW:(nn + 1) * NW],
                        start=(k == 0), stop=(k == KH - 1))
                ob = pr.tile([128, NW], F32, name=f"ob{t}_{nn}", tag="ob", bufs=4)
                nc.vector.tensor_tensor(
                    out=ob[:], in0=ps_o[:], in1=b_out[:, nn * NW:(nn + 1) * NW],
                    op=mybir.AluOpType.add)
                nc.sync.dma_start(
                    out=out_d[t * 128:(t + 1) * 128, nn * NW:(nn + 1) * NW],
                    in_=ob[:])

            for s in range(n_steps):
                g, half = s // 2, s % 2
                eT = eT_w[g]

                # ---- recurrence matmuls (lanes A: psum[0:64], B: psum[64:128])
                ps_rz = pp.tile([128, 1024], F32, name=f"psrz{s}", tag="psrz", bufs=1)
                ps_hn = pp.tile([128, 512], F32, name=f"pshn{s}", tag="pshn", bufs=1)
                ps_gin = pp.tile([128, 512], F32, name=f"psgin{s}", tag="psgin", bufs=1)

                for k in range(KH):
                    lh = hT[:, k % 4, 64 * (k // 4):64 * (k // 4) + 64]
                    st = (k == 0)
                    for (lo, wA, wB) in ((0, w_rzA, w_rzB),):
                        nc.tensor.matmul(out=ps_rz[0:64, 0:512], lhsT=lh,
                                         rhs=wA[:, k, 0:512], start=st, stop=False,
                                         skip_group_check=True)
                        nc.tensor.matmul(out=ps_rz[64:128, 0:512], lhsT=lh,
                                         rhs=wB[:, k, 0:512], start=st, stop=False,
                                         skip_group_check=True)
                        nc.tensor.matmul(out=ps_rz[0:64, 512:1024], lhsT=lh,
                                         rhs=wA[:, k, 512:1024], start=st, stop=False,
                                         skip_group_check=True)
                        nc.tensor.matmul(out=ps_rz[64:128, 512:1024], lhsT=lh,
                                         rhs=wB[:, k, 512:1024], start=st, stop=False,
                                         skip_group_check=True)
                for c in range(KE):
                    le = eT[:, c, 64 * half:64 * half + 64]
                    sp = (c == KE - 1)
                    nc.tensor.matmul(out=ps_rz[0:64, 0:512], lhsT=le,
                                     rhs=w_irzA[:, c, 0:512], start=False, stop=sp,
                                     skip_group_check=True)
                    nc.tensor.matmul(out=ps_rz[64:128, 0:512], lhsT=le,
                                     rhs=w_irzB[:, c, 0:512], start=False, stop=sp,
                                     skip_group_check=True)
                    nc.tensor.matmul(out=ps_rz[0:64, 512:1024], lhsT=le,
                                     rhs=w_irzA[:, c, 512:1024], start=False, stop=sp,
                                     skip_group_check=True)
                    nc.tensor.matmul(out=ps_rz[64:128, 512:1024], lhsT=le,
                                     rhs=w_irzB[:, c, 512:1024], start=False, stop=sp,
                                     skip_group_check=True)
                for k in range(KH):
                    lh = hT[:, k % 4, 64 * (k // 4):64 * (k // 4) + 64]
                    st, sp = (k == 0), (k == KH - 1)
                    nc.tensor.matmul(out=ps_hn[0:64, :], lhsT=lh, rhs=w_hnA[:, k, :],
                                     start=st, stop=sp, skip_group_check=True)
                    nc.tensor.matmul(out=ps_hn[64:128, :], lhsT=lh, rhs=w_hnB[:, k, :],
                                     start=st, stop=sp, skip_group_check=True)
                for c in range(KE):
                    le = eT[:, c, 64 * half:64 * half + 64]
                    st, sp = (c == 0), (c == KE - 1)
                    nc.tensor.matmul(out=ps_gin[0:64, :], lhsT=le, rhs=w_inA[:, c, :],
                                     start=st, stop=sp, skip_group_check=True)
                    nc.tensor.matmul(out=ps_gin[64:128, :], lhsT=le, rhs=w_inB[:, c, :],
                                     start=st, stop=sp, skip_group_check=True)

                # ---- gate chain (DVE/ACT) — overlaps the output jobs below
                rz = pr.tile([128, 1024], F32, name=f"rz{s}", tag="rz", bufs=2)
                nc.vector.tensor_tensor(out=ps_rz[:], in0=ps_rz[:], in1=bias_rz[:],
                                        op=mybir.AluOpType.add)
                nc.scalar.activation(out=rz[:], in_=ps_rz[:],
                                     func=mybir.ActivationFunctionType.Sigmoid)
                nc.vector.tensor_tensor(out=ps_hn[:], in0=ps_hn[:], in1=bias_nhh[:],
                                        op=mybir.AluOpType.add)
                nc.vector.tensor_tensor(out=ps_gin[:], in0=ps_gin[:], in1=bias_nih[:],
                                        op=mybir.AluOpType.add)
                tn = pr.tile([128, 512], F32, name=f"tn{s}", tag="tn", bufs=2)
                nc.vector.tensor_tensor(out=tn[:], in0=rz[:, 0:512], in1=ps_hn[:],
                                        op=mybir.AluOpType.mult)
                nc.vector.tensor_tensor(out=tn[:], in0=tn[:], in1=ps_gin[:],
                                        op=mybir.AluOpType.add)
                n_sb = pr.tile([128, 512], F32, name=f"n{s}", tag="n", bufs=2)
                nc.scalar.activation(out=n_sb[:], in_=tn[:],
                                     func=mybir.ActivationFunctionType.Tanh)
                d_sb = pr.tile([128, 512], F32, name=f"d{s}", tag="d", bufs=2)
                nc.vector.tensor_tensor(out=d_sb[:], in0=h_st[:], in1=n_sb[:],
                                        op=mybir.AluOpType.subtract)
                nc.vector.tensor_tensor(out=d_sb[:], in0=rz[:, 512:1024], in1=d_sb[:],
                                        op=mybir.AluOpType.mult)
                h_st = pr.tile([128, 512], F32, name=f"hst{s}", tag="hst", bufs=2)
                nc.vector.tensor_tensor(out=h_st[:], in0=n_sb[:], in1=d_sb[:],
                                        op=mybir.AluOpType.add)

                # ---- output jobs fill the PE while the gate chain runs
                if s >= 3:
                    for _ in range(4):
                        if jp < len(jobs) and 2 * jobs[jp][0] + 2 <= s:
                            emit_job(*jobs[jp])
                            jp += 1

                # ---- transpose h back to lhsT layout (+ fp16 copies)
                # full-partition [128,128] block transposes of the stacked h:
                # block m yields [chunk m | chunk m+4] side by side.
                h16 = pr.tile([128, 512], FP16, name=f"h16_{s}", tag="h16", bufs=2)
                nc.vector.tensor_copy(out=h16[:], in_=h_st[:])
                ps_T = pp.tile([128, 512], FP16, name=f"psT{s}", tag="psT", bufs=1)
                for m in range(4):
                    nc.tensor.matmul(
                        out=ps_T[:, 128 * m:128 * m + 128],
                        lhsT=h16[:, 128 * m:128 * m + 128],
                        rhs=ident[:],
                        is_transpose=True, start=(m == 0), stop=(m == 3),
                        skip_group_check=True)
                hT = pr.tile([128, 4, 128], FP16, name=f"hT{s}", tag="hT", bufs=2)
                nc.vector.tensor_copy(
                    out=hT[:], in_=ps_T[:].rearrange("p (m c) -> p m c", m=4))
                if half == 0:
                    hs_w[g] = pr.tile([128, KH, 128], FP16, name=f"hs{g}",
                                      tag="hs", bufs=4)
                nc.vector.tensor_copy(
                    out=hs_w[g][:, :, 64 * half:64 * half + 64],
                    in_=ps_T[:].rearrange("p (m hh b) -> p hh m b", m=4, hh=2))

                # ---- prefetch next embedding tile
                if half == 1 and g + 8 < (n_steps + 1) // 2:
                    eT_w[g + 8] = gather_tile(g + 8)

            # ---- drain remaining output jobs
            while jp < len(jobs):
                emit_job(*jobs[jp])
                jp += 1

    nc.compile()
    _CACHE[key] = nc
    return nc


def _prep_in_maps(x, hidden, emb, w_ih, w_hh, b_ih, b_hh, w_out, b_out):
    f16, f32 = np.float16, np.float32

    toks = np.concatenate([np.full((1, B), 2, dtype=np.int64),
                           np.asarray(x)[:-1].astype(np.int64)], axis=0)
    t_flat = toks.reshape(SB).astype(np.int32)
    idx = np.ascontiguousarray(t_flat.reshape(NT, 128).T)        # [128, 32]

    emb_t = np.asarray(emb, dtype=f32).copy()
    emb_t[0] = 0.0
    emb_t = np.maximum(emb_t, 0.0).astype(f16)                    # relu folded

    w_hh = np.asarray(w_hh, dtype=f32)
    w_ih = np.asarray(w_ih, dtype=f32)
    Wr, Wz, Wn = w_hh[0:H], w_hh[H:2 * H], w_hh[2 * H:3 * H]
    Ur, Uz, Un = w_ih[0:H], w_ih[H:2 * H], w_ih[2 * H:3 * H]

    def kview(m, kc):  # [rows, K] -> [128, kc, rows] fp16 (K on partitions)
        return np.ascontiguousarray(
            m.T.reshape(kc, 128, m.shape[0]).transpose(1, 0, 2)).astype(f16)

    w_rzA = kview(np.concatenate([Wr[0:512], Wz[0:512]], 0), KH)
    w_rzB = kview(np.concatenate([Wr[512:1024], Wz[512:1024]], 0), KH)
    w_hnA = kview(Wn[0:512], KH)
    w_hnB = kview(Wn[512:1024], KH)
    w_irzA = kview(np.concatenate([Ur[0:512], Uz[0:512]], 0), KE)
    w_irzB = kview(np.concatenate([Ur[512:1024], Uz[512:1024]], 0), KE)
    w_inA = kview(Un[0:512], KE)
    w_inB = kview(Un[512:1024], KE)

    b_ih = np.asarray(b_ih, dtype=f32)
    b_hh = np.asarray(b_hh, dtype=f32)
    brz = (b_ih + b_hh)
    bias_rz = np.empty((128, 1024), f32)
    bias_nhh = np.empty((128, 512), f32)
    bias_nih = np.empty((128, 512), f32)
    for hp in (0, 1):
        r = slice(64 * hp, 64 * hp + 64)
        bias_rz[r, 0:512] = brz[0:H][512 * hp:512 * hp + 512][None, :]
        bias_rz[r, 512:1024] = brz[H:2 * H][512 * hp:512 * hp + 512][None, :]
        bias_nhh[r] = b_hh[2 * H:3 * H][512 * hp:512 * hp + 512][None, :]
        bias_nih[r] = b_ih[2 * H:3 * H][512 * hp:512 * hp + 512][None, :]

    h0 = np.asarray(hidden, dtype=f32)[0]                         # [B, H]
    # hT0[p, m, 64*hh + b] = h0[b, 128*(m + 4*hh) + p]
    hT0 = np.ascontiguousarray(
        h0.T.reshape(2, 4, 128, B).transpose(2, 1, 0, 3).reshape(128, 4, 128)
    ).astype(f16)
    h0st = np.concatenate([h0[:, 0:512], h0[:, 512:1024]], axis=0).astype(f32)

    w_out = np.asarray(w_out, dtype=f32)
    b_out = np.asarray(b_out, dtype=f32)

    shared = dict(
        emb_t=emb_t, idx=idx,
        w_rzA=w_rzA, w_rzB=w_rzB, w_hnA=w_hnA, w_hnB=w_hnB,
        w_irzA=w_irzA, w_irzB=w_irzB, w_inA=w_inA, w_inB=w_inB,
        bias_rz=bias_rz, bias_nhh=bias_nhh, bias_nih=bias_nih,
        hT0=hT0, h0st=h0st,
    )
    in_maps = []
    for c in range(NCORES):
        sl = slice(c * VC, (c + 1) * VC)
        w_outT = np.ascontiguousarray(
            w_out[sl].T.reshape(KH, 128, VC).transpose(1, 0, 2)).astype(f16)
        b_out_bc = np.ascontiguousarray(
            np.broadcast_to(b_out[sl], (128, VC))).astype(f16)
        in_maps.append(dict(shared, w_outT=w_outT, b_out_bc=b_out_bc))
    return in_maps


def _assemble(results):
    full = np.concatenate(
        [r["out"].reshape(S, B, VC) for r in results], axis=2)   # (S, B, V)
    return np.ascontiguousarray(full.transpose(1, 0, 2)[None]).astype(np.float32)


def _run(trace=False, tmpdir=None, **inputs):
    nc = _build()
    in_maps = _prep_in_maps(**inputs)
    res = run_bass_kernel_spmd(nc, in_maps, list(range(NCORES)),
                               trace=trace, tmpdir=tmpdir)
    return _assemble(res.results), res


def kernel(**inputs) -> np.ndarray:
    out, _ = _run(**inputs)
    return out


if __name__ == "__main__":
    rng = np.random.default_rng(0)
    ins = dict(
        x=rng.integers(0, V, (S, B)).astype(np.int32),
        hidden=rng.standard_normal((1, B, H)).astype(np.float32),
        emb=rng.standard_normal((V, E)).astype(np.float32),
        w_ih=rng.uniform(-1 / 32, 1 / 32, (3 * H, E)).astype(np.float32),
        w_hh=rng.uniform(-1 / 32, 1 / 32, (3 * H, H)).astype(np.float32),
        b_ih=rng.uniform(-1 / 32, 1 / 32, (3 * H,)).astype(np.float32),
        b_hh=rng.uniform(-1 / 32, 1 / 32, (3 * H,)).astype(np.float32),
        w_out=rng.uniform(-1 / 32, 1 / 32, (V, H)).astype(np.float32),
        b_out=rng.uniform(-1 / 32, 1 / 32, (V,)).astype(np.float32),
    )
    out = kernel(**ins)
    print("out", out.shape, out.dtype, float(np.abs(out).max()))

